# revision 15
# baseline (speedup 1.0000x reference)
"""ContinuousDeepFM Trainium2 kernel (8-core data-parallel over batch).

Math (algebraically collapsed from the reference — the [B,D,D] interaction
tensor is never materialized):
    fo  = x @ W1 + bias
    xw  = x @ W2
    so[b,j] = 0.5 * xw[b,j]^2 * t[b],  t[b] = sum_i x[b,i]^2 - (sum_i x[b,i])^2
    h   = MLP(x @ Wf)   (3 ReLU layers + final linear, weights mlp_w[i].T)
    out = fo + so + h

Sharding: batch 512 -> 64 rows per core; weights replicated. On-chip layout
is feature-major (activations stored transposed as 4 chunks of 128
partitions) so no on-chip transposes are needed; per-feature biases become
per-partition scalars. t depends only on x, so it is computed host-side in
fp64 and shipped pre-broadcast.

Precision: the output is dominated by the second-order term (RMS ~2e5 vs
~23 for fo and ~1 for h). W2 and the x used with it run in bf16 (measured
end-to-end rel err ~3e-3 vs the 2e-2 gate); the fo/deep weights and
activations run in fp8e4m3. x is pre-cast host-side to both dtypes so no
on-chip casts gate the first matmul.

Schedule notes (from NTFF traces): the two HWDGE rings share one pool of 16
DMA engines (~205-280 GB/s aggregate), so ring assignment is about arrival
ORDER, not bandwidth. Each PSUM accumulation group lives in a single bank
as one [128, 256] tile so post-matmul elementwise work is one instruction
instead of four (per-instruction overhead ~200ns dominates 64-element
ops). The final bias (bias + mlp_b[3]) enters the last PSUM group via
rank-1 fp32 matmuls (contraction dim 1 against a ones vector), which takes
it off the Vector/GpSimd critical tail entirely; GpSimd tensor_scalar is
~1.1us/op on this part and must never sit on the tail. The output is one
128KB DMA issued as soon as the single fused add lands.
"""

import numpy as np
import ml_dtypes

B = 512
D = 512
NCORES = 8
BL = B // NCORES  # 64 batch rows per core
P = 128
KC = D // P  # 4 partition chunks of the feature dim

F8 = ml_dtypes.float8_e4m3
BF16 = ml_dtypes.bfloat16

_NC_CACHE = {}


def _split_multi_waits(nc, mybir):
    """This container's walrus build supports only ONE sync wait per
    instruction, but Tile's scheduler attaches several (e.g. the exit
    drain). Split extras into preceding single-wait NoOps on the same
    engine — in-order execution preserves the barrier semantics."""
    ctr = 0
    for fn in nc.m.functions:
        for blk in fn.blocks:
            insts = blk.instructions
            if not any(
                i.sync_info is not None
                and i.sync_info.on_wait
                and len(i.sync_info.on_wait) > 1
                for i in insts
            ):
                continue
            out = []
            for inst in insts:
                si = inst.sync_info
                if si is not None and si.on_wait and len(si.on_wait) > 1:
                    waits = list(si.on_wait)
                    for w in waits[:-1]:
                        ctr += 1
                        nop = mybir.InstNoOp(
                            name=f"wsplit-{ctr}-{inst.name}", ins=[], outs=[]
                        )
                        nop.engine = inst.engine
                        nop.sync_info = mybir.SyncInfo(on_wait=[w], on_update=[])
                        out.append(nop)
                    si.on_wait = [waits[-1]]
                out.append(inst)
            blk.instructions = out
    return ctr


def _build_nc():
    import concourse.bass as bass
    import concourse.mybir as mybir
    import concourse.tile as tile

    dt = mybir.dt
    f32 = dt.float32
    f8 = dt.float8e4
    bf = dt.bfloat16
    Alu = mybir.AluOpType
    Act = mybir.ActivationFunctionType

    nc = bass.Bass("TRN2", target_bir_lowering=False, debug=False)

    # x8 (fp8), xb (bf16), th+bias (f32) byte-packed into one tensor so the
    # whole activation side is ONE DMA: the tile framework round-robins DMAs
    # over 8 HWDGE semaphore lanes, and a 9th+ input DMA serializes behind
    # an earlier one's completion.
    ACT_BYTES = KC * BL + 2 * KC * BL + 4 * (BL + 16)
    act_d = nc.dram_tensor("act_d", [P, ACT_BYTES], dt.uint8, kind="ExternalInput")
    bt_d = nc.dram_tensor("bt_d", [1, D], f32, kind="ExternalInput")
    wf_d = nc.dram_tensor("wf_d", [P, KC * D], f8, kind="ExternalInput")
    mw_d = nc.dram_tensor("mw_d", [P, 4 * KC * D], f8, kind="ExternalInput")
    w1_d = nc.dram_tensor("w1_d", [P, KC * D], f8, kind="ExternalInput")
    w2_d = nc.dram_tensor("w2_d", [P, KC * D], bf, kind="ExternalInput")
    out_d = nc.dram_tensor("out_d", [P, KC * BL], f32, kind="ExternalOutput")

    with tile.TileContext(nc) as tc:
        with (
            tc.tile_pool(name="w", bufs=1) as wpool,
            tc.tile_pool(name="act", bufs=1) as apool,
            tc.tile_pool(name="ps", bufs=1, space="PSUM") as pspool,
        ):
            # ---- input DMAs. Ring A (sync) carries the deep-chain weights
            # in consumption order; ring B (scalar) carries activations +
            # w2/w1. The rings share DMA engines, so this is ordering only.
            wf_sb = wpool.tile([P, KC * D], f8, tag="wf")
            nc.sync.dma_start(wf_sb[:], wf_d.ap())
            mw_sb = wpool.tile([P, 4 * KC * D], f8, tag="mw")
            for i in range(4):
                nc.sync.dma_start(
                    mw_sb[:, i * KC * D : (i + 1) * KC * D],
                    mw_d.ap()[:, i * KC * D : (i + 1) * KC * D],
                )

            act_sb = apool.tile([P, ACT_BYTES], dt.uint8, tag="act")
            nc.scalar.dma_start(act_sb[:], act_d.ap())
            bt_sb = apool.tile([1, D], f32, tag="bt")
            nc.scalar.dma_start(bt_sb[:], bt_d.ap())
            w2_sb = wpool.tile([P, KC * D], bf, tag="w2")
            nc.scalar.dma_start(w2_sb[:], w2_d.ap())
            w1_sb = wpool.tile([P, KC * D], f8, tag="w1")
            nc.scalar.dma_start(w1_sb[:], w1_d.ap())

            # typed views into the packed activation tile (byte offsets:
            # x8 @ 0, xb @ 256, th @ 768, bias @ 1024)
            act_f8 = act_sb.bitcast(f8)
            act_bf = act_sb.bitcast(bf)
            act_f32 = act_sb.bitcast(f32)
            XB0 = KC * BL // 2  # xb start in bf16 elements
            TH0 = 3 * KC * BL // 4  # th start in f32 elements

            def th_ap():
                return act_f32[:, TH0 : TH0 + BL]

            def bias_col(c):
                return act_f32[:, TH0 + BL + c : TH0 + BL + c + 1]

            def x8sl(kc):
                return act_f8[:, kc * BL : (kc + 1) * BL]

            def xbsl(kc):
                return act_bf[:, XB0 + kc * BL : XB0 + (kc + 1) * BL]

            # ones vector for the rank-1 bias matmuls
            ones = apool.tile([1, BL], f32, tag="ones")
            nc.gpsimd.memset(ones[:], 1.0)

            def wsl(t, kc, jc, base=0):
                return t[:, base + kc * D + jc * P : base + kc * D + (jc + 1) * P]

            def xsl(t, kc):
                return t[:, kc * BL : (kc + 1) * BL]

            def mm_group(ps, w_t, rhs_fn, base=0, start=True, stop=True):
                # The group shares ONE PSUM bank: start zeroes the whole
                # bank, so only the group's first matmul may carry it (the
                # rest accumulate onto the zeroed bank); stop only on last.
                for kc in range(KC):
                    for jc in range(KC):
                        nc.tensor.matmul(
                            xsl(ps, jc),
                            wsl(w_t, kc, jc, base=base),
                            rhs_fn(kc),
                            start=start and (kc == 0) and (jc == 0),
                            stop=stop and (kc == KC - 1) and (jc == KC - 1),
                        )

            def psum_group(name):
                # one bank per group: [128, 256] f32 = 1KB/partition
                return pspool.tile([P, KC * BL], f32, tag="mm", bufs=8, name=name)

            # ---- deep chain (fp8): h0 = x @ Wf; single fused drain
            h_ps = psum_group("h0")
            mm_group(h_ps, wf_sb, x8sl)
            h = apool.tile([P, KC * BL], f8, tag="h0")
            nc.vector.tensor_copy(h[:], h_ps[:])

            # l0 drain: ReLU+bias, per-chunk bias => 4 ops, split V/S
            def relu_drain(ps, i):
                hn = apool.tile([P, KC * BL], f8, tag=f"h{i + 1}")
                for jc in range(KC):
                    bcol = 4 + i * KC + jc
                    if jc % 2 == 0:
                        nc.vector.tensor_scalar(
                            xsl(hn, jc),
                            xsl(ps, jc),
                            bias_col(bcol),
                            0.0,
                            op0=Alu.add,
                            op1=Alu.max,
                        )
                    else:
                        nc.scalar.activation(
                            xsl(hn, jc),
                            xsl(ps, jc),
                            Act.Relu,
                            bias=bias_col(bcol),
                        )
                return hn

            l_ps = psum_group("l0")
            mm_group(l_ps, mw_sb, lambda kc: xsl(h, kc), base=0)
            h = relu_drain(l_ps, 0)

            h1 = h
            l_ps = psum_group("l1")
            mm_group(l_ps, mw_sb, lambda kc: xsl(h1, kc), base=KC * D)
            h = relu_drain(l_ps, 1)

            # ---- xw = x @ W2 (bf16); square in ONE Scalar op, *th on GpSimd
            xw_ps = psum_group("xw")
            mm_group(xw_ps, w2_sb, xbsl)
            xwsq = apool.tile([P, KC * BL], f32, tag="xwsq")
            nc.scalar.square(xwsq[:], xw_ps[:])
            so = apool.tile([P, KC * BL], f32, tag="so")
            for jc in range(KC):
                nc.gpsimd.tensor_mul(xsl(so, jc), xsl(xwsq, jc), th_ap())

            h2 = h
            l_ps = psum_group("l2")
            mm_group(l_ps, mw_sb, lambda kc: xsl(h2, kc), base=2 * KC * D)
            h = relu_drain(l_ps, 2)

            # ---- final PSUM group: o = x@W1 + btot (rank-1) + h3@mw[3].T
            o_ps = psum_group("o")
            mm_group(o_ps, w1_sb, x8sl, start=True, stop=False)
            for jc in range(KC):
                nc.tensor.matmul(
                    xsl(o_ps, jc),
                    bt_sb[0:1, jc * P : (jc + 1) * P],
                    ones[0:1, :],
                    start=False,
                    stop=False,
                )
            h3 = h
            mm_group(o_ps, mw_sb, lambda kc: xsl(h3, kc), base=3 * KC * D,
                     start=False, stop=True)

            # out = o + so in ONE Vector op, then one 128KB output DMA
            out_sb = apool.tile([P, KC * BL], f32, tag="out")
            nc.vector.tensor_add(out_sb[:], o_ps[:], so[:])
            nc.scalar.dma_start(out_d.ap(), out_sb[:])

    _split_multi_waits(nc, mybir)
    return nc


def _get_nc():
    if "nc" not in _NC_CACHE:
        _NC_CACHE["nc"] = _build_nc()
    return _NC_CACHE["nc"]


def _chunk_major(w):
    """[D, D] lhsT-layout weight -> dense [128, KC*D] chunk-major array."""
    return np.ascontiguousarray(
        w.reshape(KC, P, D).transpose(1, 0, 2).reshape(P, KC * D)
    )


def prepare_in_maps(inputs):
    x = np.asarray(inputs["x"], np.float32)
    w1 = np.asarray(inputs["first_order_weights"], np.float32)
    bias = np.asarray(inputs["bias"], np.float32)
    w2 = np.asarray(inputs["second_order_weights"], np.float32)
    wf = np.asarray(inputs["feature_weights"], np.float32)
    mw = np.asarray(inputs["mlp_w"], np.float32)
    mb = np.asarray(inputs["mlp_b"], np.float32)

    # t[b] = sum x^2 - (sum x)^2 (host, fp64), shipped as 0.5*t broadcast
    xd = x.astype(np.float64)
    t = (xd * xd).sum(1) - xd.sum(1) ** 2
    th_full = (0.5 * t).astype(np.float32)

    w2_dev = _chunk_major(w2).astype(BF16)
    wf_dev = _chunk_major(wf).astype(F8)
    w1_dev = _chunk_major(w1).astype(F8)
    # mw[i].T is the lhsT; layer-major, then chunk-major within each layer
    mwT = mw.transpose(0, 2, 1)  # [4, D(k), D(m)]
    mw_dev = np.ascontiguousarray(
        mwT.reshape(4, KC, P, D).transpose(2, 0, 1, 3).reshape(P, 4 * KC * D)
    ).astype(F8)
    # final bias (bias + mlp_b[3]) as a [1, D] row for the rank-1 matmuls,
    # in chunk-major feature order to match the output layout
    btot = (bias + mb[3]).astype(np.float32).reshape(1, D)
    # per-layer MLP biases as per-partition columns: [mb0(4) | mb1(4) | mb2(4)]
    mb3 = mb[:3].astype(np.float32).reshape(3, KC, P).transpose(2, 0, 1).reshape(P, 12)
    bias_dev = np.concatenate([np.zeros((P, 4), np.float32), mb3], axis=1)

    in_maps = []
    for c in range(NCORES):
        xs = x[c * BL : (c + 1) * BL, :].T  # [512, 64]
        x_dev = np.ascontiguousarray(
            xs.reshape(KC, P, BL).transpose(1, 0, 2).reshape(P, KC * BL)
        )
        thb_dev = np.concatenate(
            [
                np.broadcast_to(th_full[c * BL : (c + 1) * BL], (P, BL)),
                bias_dev,
            ],
            axis=1,
        )
        # byte-pack: x8 fp8 | xb bf16 | th f32 | bias f32
        act_dev = np.ascontiguousarray(
            np.concatenate(
                [
                    x_dev.astype(F8).view(np.uint8),
                    x_dev.astype(BF16).view(np.uint8),
                    thb_dev.astype(np.float32).view(np.uint8),
                ],
                axis=1,
            )
        )
        in_maps.append(
            {
                "act_d": act_dev,
                "bt_d": btot,
                "wf_d": wf_dev,
                "mw_d": mw_dev,
                "w1_d": w1_dev,
                "w2_d": w2_dev,
            }
        )
    return in_maps


def assemble_output(results):
    out = np.empty((B, D), np.float32)
    for c in range(NCORES):
        od = results[c]["out_d"]  # [128, KC*BL]
        outT = od.reshape(P, KC, BL).transpose(1, 0, 2).reshape(D, BL)
        out[c * BL : (c + 1) * BL, :] = outT.T
    return out


def kernel(**inputs):
    from concourse.bass_utils import run_bass_kernel_spmd

    nc = _get_nc()
    in_maps = prepare_in_maps(inputs)
    res = run_bass_kernel_spmd(nc, in_maps, core_ids=list(range(NCORES)))
    return assemble_output(res.results)


# revision 16
# speedup vs baseline: 1.0103x; 1.0103x over previous
"""ContinuousDeepFM Trainium2 kernel (8-core data-parallel over batch).

Math (algebraically collapsed from the reference — the [B,D,D] interaction
tensor is never materialized):
    fo  = x @ W1 + bias
    xw  = x @ W2
    so[b,j] = 0.5 * xw[b,j]^2 * t[b],  t[b] = sum_i x[b,i]^2 - (sum_i x[b,i])^2
    h   = MLP(x @ Wf)   (3 ReLU layers + final linear, weights mlp_w[i].T)
    out = fo + so + h

Sharding: batch 512 -> 64 rows per core; weights replicated. On-chip layout
is feature-major (activations stored transposed as 4 chunks of 128
partitions) so no on-chip transposes are needed; per-feature biases become
per-partition scalars. t depends only on x, so it is computed host-side in
fp64 and shipped pre-broadcast.

Precision: the output is dominated by the second-order term (RMS ~2e5 vs
~23 for fo and ~1 for h). W2 and the x used with it run in bf16 (measured
end-to-end rel err ~3e-3 vs the 2e-2 gate); the fo/deep weights and
activations run in fp8e4m3. x is pre-cast host-side to both dtypes so no
on-chip casts gate the first matmul.

Schedule notes (from NTFF traces): the two HWDGE rings share one pool of 16
DMA engines (~205-280 GB/s aggregate), so ring assignment is about arrival
ORDER, not bandwidth. Each PSUM accumulation group lives in a single bank
as one [128, 256] tile so post-matmul elementwise work is one instruction
instead of four (per-instruction overhead ~200ns dominates 64-element
ops). The final bias (bias + mlp_b[3]) enters the last PSUM group via
rank-1 fp32 matmuls (contraction dim 1 against a ones vector), which takes
it off the Vector/GpSimd critical tail entirely; GpSimd tensor_scalar is
~1.1us/op on this part and must never sit on the tail. The output is one
128KB DMA issued as soon as the single fused add lands.
"""

import numpy as np
import ml_dtypes

B = 512
D = 512
NCORES = 8
BL = B // NCORES  # 64 batch rows per core
P = 128
KC = D // P  # 4 partition chunks of the feature dim

F8 = ml_dtypes.float8_e4m3
BF16 = ml_dtypes.bfloat16

_NC_CACHE = {}


def _split_multi_waits(nc, mybir):
    """This container's walrus build supports only ONE sync wait per
    instruction, but Tile's scheduler attaches several (e.g. the exit
    drain). Split extras into preceding single-wait NoOps on the same
    engine — in-order execution preserves the barrier semantics."""
    ctr = 0
    for fn in nc.m.functions:
        for blk in fn.blocks:
            insts = blk.instructions
            if not any(
                i.sync_info is not None
                and i.sync_info.on_wait
                and len(i.sync_info.on_wait) > 1
                for i in insts
            ):
                continue
            out = []
            for inst in insts:
                si = inst.sync_info
                if si is not None and si.on_wait and len(si.on_wait) > 1:
                    waits = list(si.on_wait)
                    for w in waits[:-1]:
                        ctr += 1
                        nop = mybir.InstNoOp(
                            name=f"wsplit-{ctr}-{inst.name}", ins=[], outs=[]
                        )
                        nop.engine = inst.engine
                        nop.sync_info = mybir.SyncInfo(on_wait=[w], on_update=[])
                        out.append(nop)
                    si.on_wait = [waits[-1]]
                out.append(inst)
            blk.instructions = out
    return ctr


def _build_nc():
    import concourse.bass as bass
    import concourse.mybir as mybir
    import concourse.tile as tile

    dt = mybir.dt
    f32 = dt.float32
    f8 = dt.float8e4
    bf = dt.bfloat16
    Alu = mybir.AluOpType
    Act = mybir.ActivationFunctionType

    nc = bass.Bass("TRN2", target_bir_lowering=False, debug=False)

    # x8 (fp8), xb (bf16), th+bias (f32) byte-packed into one tensor so the
    # whole activation side is ONE DMA: the tile framework round-robins DMAs
    # over 8 HWDGE semaphore lanes, and a 9th+ input DMA serializes behind
    # an earlier one's completion.
    ACT_BYTES = KC * BL + 2 * KC * BL + 4 * (BL + 16)
    act_d = nc.dram_tensor("act_d", [P, ACT_BYTES], dt.uint8, kind="ExternalInput")
    bt_d = nc.dram_tensor("bt_d", [1, D], f32, kind="ExternalInput")
    wf_d = nc.dram_tensor("wf_d", [P, KC * D], f8, kind="ExternalInput")
    mw_d = nc.dram_tensor("mw_d", [P, 4 * KC * D], f8, kind="ExternalInput")
    w1_d = nc.dram_tensor("w1_d", [P, KC * D], f8, kind="ExternalInput")
    w2_d = nc.dram_tensor("w2_d", [P, KC * D], bf, kind="ExternalInput")
    out_d = nc.dram_tensor("out_d", [P, KC * BL], f32, kind="ExternalOutput")

    with tile.TileContext(nc) as tc:
        with (
            tc.tile_pool(name="w", bufs=1) as wpool,
            tc.tile_pool(name="act", bufs=1) as apool,
            tc.tile_pool(name="ps", bufs=1, space="PSUM") as pspool,
        ):
            # ---- input DMAs. Ring A (sync) carries the deep-chain weights
            # in consumption order; ring B (scalar) carries activations +
            # w2/w1. The rings share DMA engines, so this is ordering only.
            wf_sb = wpool.tile([P, KC * D], f8, tag="wf")
            nc.sync.dma_start(wf_sb[:], wf_d.ap())
            mw_sb = wpool.tile([P, 4 * KC * D], f8, tag="mw")
            for i in range(3):
                nc.sync.dma_start(
                    mw_sb[:, i * KC * D : (i + 1) * KC * D],
                    mw_d.ap()[:, i * KC * D : (i + 1) * KC * D],
                )

            act_sb = apool.tile([P, ACT_BYTES], dt.uint8, tag="act")
            nc.scalar.dma_start(act_sb[:], act_d.ap())
            bt_sb = apool.tile([1, D], f32, tag="bt")
            nc.scalar.dma_start(bt_sb[:], bt_d.ap())
            w2_sb = wpool.tile([P, KC * D], bf, tag="w2")
            nc.scalar.dma_start(w2_sb[:], w2_d.ap())
            w1_sb = wpool.tile([P, KC * D], f8, tag="w1")
            nc.scalar.dma_start(w1_sb[:], w1_d.ap())
            # mw3 rides ring B last: keeps both rings streaming to the end
            # (ring A alone tops out ~140GB/s; the engine pool does ~2x)
            nc.scalar.dma_start(
                mw_sb[:, 3 * KC * D : 4 * KC * D],
                mw_d.ap()[:, 3 * KC * D : 4 * KC * D],
            )

            # typed views into the packed activation tile (byte offsets:
            # x8 @ 0, xb @ 256, th @ 768, bias @ 1024)
            act_f8 = act_sb.bitcast(f8)
            act_bf = act_sb.bitcast(bf)
            act_f32 = act_sb.bitcast(f32)
            XB0 = KC * BL // 2  # xb start in bf16 elements
            TH0 = 3 * KC * BL // 4  # th start in f32 elements

            def th_ap():
                return act_f32[:, TH0 : TH0 + BL]

            def bias_col(c):
                return act_f32[:, TH0 + BL + c : TH0 + BL + c + 1]

            def x8sl(kc):
                return act_f8[:, kc * BL : (kc + 1) * BL]

            def xbsl(kc):
                return act_bf[:, XB0 + kc * BL : XB0 + (kc + 1) * BL]

            # ones vector for the rank-1 bias matmuls
            ones = apool.tile([1, BL], f32, tag="ones")
            nc.gpsimd.memset(ones[:], 1.0)

            def wsl(t, kc, jc, base=0):
                return t[:, base + kc * D + jc * P : base + kc * D + (jc + 1) * P]

            def xsl(t, kc):
                return t[:, kc * BL : (kc + 1) * BL]

            def mm_group(ps, w_t, rhs_fn, base=0, start=True, stop=True):
                # The group shares ONE PSUM bank: start zeroes the whole
                # bank, so only the group's first matmul may carry it (the
                # rest accumulate onto the zeroed bank); stop only on last.
                for kc in range(KC):
                    for jc in range(KC):
                        nc.tensor.matmul(
                            xsl(ps, jc),
                            wsl(w_t, kc, jc, base=base),
                            rhs_fn(kc),
                            start=start and (kc == 0) and (jc == 0),
                            stop=stop and (kc == KC - 1) and (jc == KC - 1),
                        )

            def psum_group(name):
                # one bank per group: [128, 256] f32 = 1KB/partition
                return pspool.tile([P, KC * BL], f32, tag="mm", bufs=8, name=name)

            # ---- deep chain (fp8): h0 = x @ Wf; single fused drain
            h_ps = psum_group("h0")
            mm_group(h_ps, wf_sb, x8sl)
            h = apool.tile([P, KC * BL], f8, tag="h0")
            nc.vector.tensor_copy(h[:], h_ps[:])

            # l0 drain: ReLU+bias, per-chunk bias => 4 ops, split V/S
            def relu_drain(ps, i):
                hn = apool.tile([P, KC * BL], f8, tag=f"h{i + 1}")
                for jc in range(KC):
                    bcol = 4 + i * KC + jc
                    if jc % 2 == 0:
                        nc.vector.tensor_scalar(
                            xsl(hn, jc),
                            xsl(ps, jc),
                            bias_col(bcol),
                            0.0,
                            op0=Alu.add,
                            op1=Alu.max,
                        )
                    else:
                        nc.scalar.activation(
                            xsl(hn, jc),
                            xsl(ps, jc),
                            Act.Relu,
                            bias=bias_col(bcol),
                        )
                return hn

            l_ps = psum_group("l0")
            mm_group(l_ps, mw_sb, lambda kc: xsl(h, kc), base=0)
            h = relu_drain(l_ps, 0)

            h1 = h
            l_ps = psum_group("l1")
            mm_group(l_ps, mw_sb, lambda kc: xsl(h1, kc), base=KC * D)
            h = relu_drain(l_ps, 1)

            # ---- xw = x @ W2 (bf16); square in ONE Scalar op, *th on GpSimd
            xw_ps = psum_group("xw")
            mm_group(xw_ps, w2_sb, xbsl)
            xwsq = apool.tile([P, KC * BL], f32, tag="xwsq")
            nc.scalar.square(xwsq[:], xw_ps[:])
            so = apool.tile([P, KC * BL], f32, tag="so")
            for jc in range(KC):
                nc.gpsimd.tensor_mul(xsl(so, jc), xsl(xwsq, jc), th_ap())

            h2 = h
            l_ps = psum_group("l2")
            mm_group(l_ps, mw_sb, lambda kc: xsl(h2, kc), base=2 * KC * D)
            h = relu_drain(l_ps, 2)

            # ---- final PSUM group: o = x@W1 + btot (rank-1) + h3@mw[3].T
            o_ps = psum_group("o")
            mm_group(o_ps, w1_sb, x8sl, start=True, stop=False)
            for jc in range(KC):
                nc.tensor.matmul(
                    xsl(o_ps, jc),
                    bt_sb[0:1, jc * P : (jc + 1) * P],
                    ones[0:1, :],
                    start=False,
                    stop=False,
                )
            h3 = h
            mm_group(o_ps, mw_sb, lambda kc: xsl(h3, kc), base=3 * KC * D,
                     start=False, stop=True)

            # out = o + so in ONE Vector op, then one 128KB output DMA
            out_sb = apool.tile([P, KC * BL], f32, tag="out")
            nc.vector.tensor_add(out_sb[:], o_ps[:], so[:])
            nc.scalar.dma_start(out_d.ap(), out_sb[:])

    _split_multi_waits(nc, mybir)
    return nc


def _get_nc():
    if "nc" not in _NC_CACHE:
        _NC_CACHE["nc"] = _build_nc()
    return _NC_CACHE["nc"]


def _chunk_major(w):
    """[D, D] lhsT-layout weight -> dense [128, KC*D] chunk-major array."""
    return np.ascontiguousarray(
        w.reshape(KC, P, D).transpose(1, 0, 2).reshape(P, KC * D)
    )


def prepare_in_maps(inputs):
    x = np.asarray(inputs["x"], np.float32)
    w1 = np.asarray(inputs["first_order_weights"], np.float32)
    bias = np.asarray(inputs["bias"], np.float32)
    w2 = np.asarray(inputs["second_order_weights"], np.float32)
    wf = np.asarray(inputs["feature_weights"], np.float32)
    mw = np.asarray(inputs["mlp_w"], np.float32)
    mb = np.asarray(inputs["mlp_b"], np.float32)

    # t[b] = sum x^2 - (sum x)^2 (host, fp64), shipped as 0.5*t broadcast
    xd = x.astype(np.float64)
    t = (xd * xd).sum(1) - xd.sum(1) ** 2
    th_full = (0.5 * t).astype(np.float32)

    w2_dev = _chunk_major(w2).astype(BF16)
    wf_dev = _chunk_major(wf).astype(F8)
    w1_dev = _chunk_major(w1).astype(F8)
    # mw[i].T is the lhsT; layer-major, then chunk-major within each layer
    mwT = mw.transpose(0, 2, 1)  # [4, D(k), D(m)]
    mw_dev = np.ascontiguousarray(
        mwT.reshape(4, KC, P, D).transpose(2, 0, 1, 3).reshape(P, 4 * KC * D)
    ).astype(F8)
    # final bias (bias + mlp_b[3]) as a [1, D] row for the rank-1 matmuls,
    # in chunk-major feature order to match the output layout
    btot = (bias + mb[3]).astype(np.float32).reshape(1, D)
    # per-layer MLP biases as per-partition columns: [mb0(4) | mb1(4) | mb2(4)]
    mb3 = mb[:3].astype(np.float32).reshape(3, KC, P).transpose(2, 0, 1).reshape(P, 12)
    bias_dev = np.concatenate([np.zeros((P, 4), np.float32), mb3], axis=1)

    in_maps = []
    for c in range(NCORES):
        xs = x[c * BL : (c + 1) * BL, :].T  # [512, 64]
        x_dev = np.ascontiguousarray(
            xs.reshape(KC, P, BL).transpose(1, 0, 2).reshape(P, KC * BL)
        )
        thb_dev = np.concatenate(
            [
                np.broadcast_to(th_full[c * BL : (c + 1) * BL], (P, BL)),
                bias_dev,
            ],
            axis=1,
        )
        # byte-pack: x8 fp8 | xb bf16 | th f32 | bias f32
        act_dev = np.ascontiguousarray(
            np.concatenate(
                [
                    x_dev.astype(F8).view(np.uint8),
                    x_dev.astype(BF16).view(np.uint8),
                    thb_dev.astype(np.float32).view(np.uint8),
                ],
                axis=1,
            )
        )
        in_maps.append(
            {
                "act_d": act_dev,
                "bt_d": btot,
                "wf_d": wf_dev,
                "mw_d": mw_dev,
                "w1_d": w1_dev,
                "w2_d": w2_dev,
            }
        )
    return in_maps


def assemble_output(results):
    out = np.empty((B, D), np.float32)
    for c in range(NCORES):
        od = results[c]["out_d"]  # [128, KC*BL]
        outT = od.reshape(P, KC, BL).transpose(1, 0, 2).reshape(D, BL)
        out[c * BL : (c + 1) * BL, :] = outT.T
    return out


def kernel(**inputs):
    from concourse.bass_utils import run_bass_kernel_spmd

    nc = _get_nc()
    in_maps = prepare_in_maps(inputs)
    res = run_bass_kernel_spmd(nc, in_maps, core_ids=list(range(NCORES)))
    return assemble_output(res.results)


# revision 53
# speedup vs baseline: 1.0442x; 1.0336x over previous
"""ContinuousDeepFM Trainium2 kernel (8-core data-parallel over batch).

Math (algebraically collapsed from the reference — the [B,D,D] interaction
tensor is never materialized):
    fo  = x @ W1 + bias
    xw  = x @ W2
    so[b,j] = 0.5 * xw[b,j]^2 * t[b],  t[b] = sum_i x[b,i]^2 - (sum_i x[b,i])^2
    h   = MLP(x @ Wf)   (3 ReLU layers + final linear, weights mlp_w[i].T)
    out = fo + so + h

Sharding: batch 512 -> 64 rows per core; weights replicated. On-chip layout
is feature-major (activations stored transposed as 4 chunks of 128
partitions) so no on-chip transposes are needed. t depends only on x, so it
is computed host-side in fp64 and shipped pre-broadcast.

Precision: the output is dominated by the second-order term (RMS ~2e5 vs
~23 for fo and ~1 for h). W2 and the x used with it run in bf16 (end-to-end
rel err 2.7e-3 vs the 2e-2 gate); the deep/first-order weights and
activations run in fp8e4m3; x is pre-cast host-side to both dtypes. All
biases ship as fp8 rows and enter each layer's PSUM via rank-1 matmuls
against a ones vector (~50ns each; fp32 matmuls would LOW/HIGH double-pass
at ~750ns), so drains are bias-free.

Schedule (from NTFF trace iteration):
- Both HWDGE rings share one pool of 16 DMA engines; ring choice is about
  ORDER/completion-gating, not bandwidth. >8 total DMAs round-robin onto 8
  semaphore lanes and serialize, so the input side is exactly 8 DMAs.
- x8|xb|th|bias|wf byte-packed into one 3136B-row tensor (h0's entire gate
  is one early DMA; thin-row tensors starve behind fat ones, so they must
  not sit mid-ring). Ring A: actwf, mw1, mw2, w1; ring B: bt, mw0 (l0
  ungates right after h0), w2, mw3 — per-layer mw DMAs land just ahead of
  their layer; each ring's last DMA feeds only tail work (the straggler
  DMA engine adds ~0.5-1us to every completion).
- Each PSUM accumulation group is ONE [128,256] tile in one bank (start
  zeroes the whole bank, so only a group's first matmul carries it) and
  matmuls are emitted in a diagonal (kc+jc) wavefront: output slices finish
  early, input chunks are first read late, and layer drains overlap the
  group. Drains are single wide Vector ReLU ops — the Tile scheduler
  chains multi-op drains ~320ns apart via transitive semaphore elision.
- xw + its square (Scalar) + *t (GpSimd) run mid-chain; the final PSUM
  group (btot + x@W1 + h3@mw3) closes right after the last weights land;
  out = o + so as two half adds, each half DMA'd on its own ring.
"""

import numpy as np
import ml_dtypes

B = 512
D = 512
NCORES = 8
BL = B // NCORES  # 64 batch rows per core
P = 128
KC = D // P  # 4 partition chunks of the feature dim

F8 = ml_dtypes.float8_e4m3
BF16 = ml_dtypes.bfloat16

_NC_CACHE = {}


def _split_multi_waits(nc, mybir):
    """This container's walrus build supports only ONE sync wait per
    instruction, but Tile's scheduler attaches several (e.g. the exit
    drain). Split extras into preceding single-wait NoOps on the same
    engine — in-order execution preserves the barrier semantics."""
    ctr = 0
    for fn in nc.m.functions:
        for blk in fn.blocks:
            insts = blk.instructions
            if not any(
                i.sync_info is not None
                and i.sync_info.on_wait
                and len(i.sync_info.on_wait) > 1
                for i in insts
            ):
                continue
            out = []
            for inst in insts:
                si = inst.sync_info
                if si is not None and si.on_wait and len(si.on_wait) > 1:
                    waits = list(si.on_wait)
                    for w in waits[:-1]:
                        ctr += 1
                        nop = mybir.InstNoOp(
                            name=f"wsplit-{ctr}-{inst.name}", ins=[], outs=[]
                        )
                        nop.engine = inst.engine
                        nop.sync_info = mybir.SyncInfo(on_wait=[w], on_update=[])
                        out.append(nop)
                    si.on_wait = [waits[-1]]
                out.append(inst)
            blk.instructions = out
    return ctr


def _build_nc():
    import concourse.bass as bass
    import concourse.mybir as mybir
    import concourse.tile as tile

    dt = mybir.dt
    f32 = dt.float32
    f8 = dt.float8e4
    bf = dt.bfloat16
    Alu = mybir.AluOpType
    Act = mybir.ActivationFunctionType

    nc = bass.Bass("TRN2", target_bir_lowering=False, debug=False)

    # x8 (fp8), xb (bf16), th+bias (f32) byte-packed into one tensor so the
    # whole activation side is ONE DMA: the tile framework round-robins DMAs
    # over 8 HWDGE semaphore lanes, and a 9th+ input DMA serializes behind
    # an earlier one's completion.
    # x8|xb|th|bias|wf packed into one 3136B-row tensor: h0 gates on this
    # single early DMA, and the rows are fat enough to stream well
    ACT_BYTES = KC * BL + 2 * KC * BL + 4 * (BL + 16) + KC * D
    act_d = nc.dram_tensor("act_d", [P, ACT_BYTES], dt.uint8, kind="ExternalInput")
    # ALL biases as fp8 rows [mb0|mb1|mb2|btot] (values ~±0.04; fp8
    # quantization error ~2e-3 absolute is invisible next to the ~2e5-RMS
    # output). They enter each layer's PSUM via rank-1 fp8 matmuls (~50ns
    # each), so the ReLU drains carry no bias and collapse to two wide ops
    # per layer — the Tile scheduler chains drains ~320ns apart, so fewer
    # drain ops directly shortens every layer transition.
    bt_d = nc.dram_tensor("bt_d", [1, 4 * D], f8, kind="ExternalInput")
    w1_d = nc.dram_tensor("w1_d", [P, KC * D], f8, kind="ExternalInput")
    mw_d = nc.dram_tensor("mw_d", [P, 4 * KC * D], f8, kind="ExternalInput")
    w2_d = nc.dram_tensor("w2_d", [P, KC * D], bf, kind="ExternalInput")
    out_d = nc.dram_tensor("out_d", [P, KC * BL], f32, kind="ExternalOutput")

    with tile.TileContext(nc) as tc:
        with (
            tc.tile_pool(name="w", bufs=1) as wpool,
            tc.tile_pool(name="act", bufs=1) as apool,
            tc.tile_pool(name="ps", bufs=1, space="PSUM") as pspool,
        ):
            # ---- input DMAs. Ring A (sync) carries the deep-chain weights
            # in consumption order; ring B (scalar) carries activations +
            # w2/w1. The rings share DMA engines, so this is ordering only.
            # Ring A (sync): act+wf packed (h0's whole gate), then mw1,
            # mw2, w1 in consumption order. Ring B (scalar): bt, mw0 (l0
            # ungates right after h0), w2, mw3.
            act_sb = apool.tile([P, ACT_BYTES], dt.uint8, tag="act")
            nc.sync.dma_start(act_sb[:], act_d.ap())
            mw_sb = wpool.tile([P, 4 * KC * D], f8, tag="mw")
            for i in (1, 2):
                nc.sync.dma_start(
                    mw_sb[:, i * KC * D : (i + 1) * KC * D],
                    mw_d.ap()[:, i * KC * D : (i + 1) * KC * D],
                )
            w1_sb = wpool.tile([P, KC * D], f8, tag="w1")
            nc.sync.dma_start(w1_sb[:], w1_d.ap())

            bt_sb = apool.tile([1, 4 * D], f8, tag="bt")
            nc.scalar.dma_start(bt_sb[:], bt_d.ap())
            nc.scalar.dma_start(
                mw_sb[:, 0 : KC * D], mw_d.ap()[:, 0 : KC * D]
            )
            w2_sb = wpool.tile([P, KC * D], bf, tag="w2")
            nc.scalar.dma_start(w2_sb[:], w2_d.ap())
            nc.scalar.dma_start(
                mw_sb[:, 3 * KC * D : 4 * KC * D],
                mw_d.ap()[:, 3 * KC * D : 4 * KC * D],
            )

            # typed views into the packed tile (byte offsets: x8 @ 0,
            # xb @ 256, th @ 768, bias @ 1024, wf @ 1088)
            act_f8 = act_sb.bitcast(f8)
            act_bf = act_sb.bitcast(bf)
            act_f32 = act_sb.bitcast(f32)
            XB0 = KC * BL // 2  # xb start in bf16 elements
            TH0 = 3 * KC * BL // 4  # th start in f32 elements
            WF0 = KC * BL + 2 * KC * BL + 4 * (BL + 16)  # wf byte offset

            def th_ap():
                return act_f32[:, TH0 : TH0 + BL]

            def x8sl(kc):
                return act_f8[:, kc * BL : (kc + 1) * BL]

            def xbsl(kc):
                return act_bf[:, XB0 + kc * BL : XB0 + (kc + 1) * BL]

            # ones vector for the rank-1 bias matmuls
            ones = apool.tile([1, BL], f8, tag="ones")
            nc.gpsimd.memset(ones[:], 1.0)

            def wsl(t, kc, jc, base=0):
                return t[:, base + kc * D + jc * P : base + kc * D + (jc + 1) * P]

            def xsl(t, kc):
                return t[:, kc * BL : (kc + 1) * BL]

            # Diagonal wavefront (kc+jc ascending): input chunk kc is first
            # read at instruction ~kc(kc+1)/2 (late), while output slice jc
            # completes at instruction ~10+jc*3 (early) — so each layer's
            # drains overlap the group and the next layer never bubbles on
            # a drain that hasn't fired.
            DIAG = [
                (kc, w - kc)
                for w in range(2 * KC - 1)
                for kc in range(max(0, w - KC + 1), min(KC, w + 1))
            ]

            def mm_group(ps, w_t, rhs_fn, base=0, start=True, stop=True):
                # The group shares ONE PSUM bank: start zeroes the whole
                # bank, so only the group's first matmul may carry it (the
                # rest accumulate onto the zeroed bank); stop only on last.
                for n, (kc, jc) in enumerate(DIAG):
                    nc.tensor.matmul(
                        xsl(ps, jc),
                        wsl(w_t, kc, jc, base=base),
                        rhs_fn(kc),
                        start=start and n == 0,
                        stop=stop and n == len(DIAG) - 1,
                    )

            def psum_group(name):
                # one bank per group: [128, 256] f32 = 1KB/partition
                return pspool.tile([P, KC * BL], f32, tag="mm", bufs=8, name=name)

            # ---- deep chain (fp8): h0 = x @ Wf; single fused drain
            h_ps = psum_group("h0")
            mm_group(h_ps, act_f8, x8sl, base=WF0)
            h = apool.tile([P, KC * BL], f8, tag="h0")
            nc.vector.tensor_copy(h[:], h_ps[:])

            def bias_mms(ps, row, start=False):
                # rank-1 fp8 matmuls: add bias row `row` of bt_sb into every
                # slice of the group's PSUM bank before the weight matmuls
                for jc in range(KC):
                    nc.tensor.matmul(
                        xsl(ps, jc),
                        bt_sb[0:1, row * D + jc * P : row * D + (jc + 1) * P],
                        ones[0:1, :],
                        start=start and jc == 0,
                        stop=False,
                    )

            # drains are bias-free (bias lives in PSUM): ONE wide ReLU op
            # on Vector per layer — every extra drain op adds a ~320ns
            # scheduler-chained hop to the layer transition
            def relu_drain(ps, i):
                hn = apool.tile([P, KC * BL], f8, tag=f"h{i + 1}")
                nc.vector.tensor_scalar(hn[:], ps[:], 0.0, None, op0=Alu.max)
                return hn

            l_ps = psum_group("l0")
            bias_mms(l_ps, 0, start=True)
            mm_group(l_ps, mw_sb, lambda kc: xsl(h, kc), base=0, start=False)
            h1 = relu_drain(l_ps, 0)

            l_ps = psum_group("l1")
            bias_mms(l_ps, 1, start=True)
            mm_group(l_ps, mw_sb, lambda kc: xsl(h1, kc), base=KC * D,
                     start=False)
            h2 = relu_drain(l_ps, 1)

            # ---- xw = x @ W2 (bf16) slots behind l1 (w2 lands mid-chain).
            # Square in ONE Scalar op, *th on GpSimd — all done long before
            # the tail.
            xw_ps = psum_group("xw")
            mm_group(xw_ps, w2_sb, xbsl)
            xwsq = apool.tile([P, KC * BL], f32, tag="xwsq")
            nc.scalar.square(xwsq[:], xw_ps[:])
            so = apool.tile([P, KC * BL], f32, tag="so")
            for jc in range(KC):
                nc.gpsimd.tensor_mul(xsl(so, jc), xsl(xwsq, jc), th_ap())




            l_ps = psum_group("l2")
            bias_mms(l_ps, 2, start=True)
            mm_group(l_ps, mw_sb, lambda kc: xsl(h2, kc), base=2 * KC * D,
                     start=False)
            h3 = relu_drain(l_ps, 2)

            # ---- final PSUM group: btot + x@W1, then += h3 @ mw[3].T
            o_ps = psum_group("o")
            bias_mms(o_ps, 3, start=True)
            mm_group(o_ps, w1_sb, x8sl, start=False, stop=False)
            mm_group(o_ps, mw_sb, lambda kc: xsl(h3, kc), base=3 * KC * D,
                     start=False, stop=True)

            # out = o + so as two half adds, each half DMA'd on its own
            # ring as soon as its add lands
            out_sb = apool.tile([P, KC * BL], f32, tag="out")
            HO = KC * BL // 2
            nc.vector.tensor_add(out_sb[:, 0:HO], o_ps[:, 0:HO], so[:, 0:HO])
            nc.scalar.dma_start(out_d.ap()[:, 0:HO], out_sb[:, 0:HO])
            nc.vector.tensor_add(
                out_sb[:, HO : 2 * HO], o_ps[:, HO : 2 * HO], so[:, HO : 2 * HO]
            )
            nc.sync.dma_start(out_d.ap()[:, HO : 2 * HO], out_sb[:, HO : 2 * HO])

    _split_multi_waits(nc, mybir)
    return nc


def _get_nc():
    if "nc" not in _NC_CACHE:
        _NC_CACHE["nc"] = _build_nc()
    return _NC_CACHE["nc"]


def _chunk_major(w):
    """[D, D] lhsT-layout weight -> dense [128, KC*D] chunk-major array."""
    return np.ascontiguousarray(
        w.reshape(KC, P, D).transpose(1, 0, 2).reshape(P, KC * D)
    )


def prepare_in_maps(inputs):
    x = np.asarray(inputs["x"], np.float32)
    w1 = np.asarray(inputs["first_order_weights"], np.float32)
    bias = np.asarray(inputs["bias"], np.float32)
    w2 = np.asarray(inputs["second_order_weights"], np.float32)
    wf = np.asarray(inputs["feature_weights"], np.float32)
    mw = np.asarray(inputs["mlp_w"], np.float32)
    mb = np.asarray(inputs["mlp_b"], np.float32)

    # t[b] = sum x^2 - (sum x)^2 (host, fp64), shipped as 0.5*t broadcast
    xd = x.astype(np.float64)
    t = (xd * xd).sum(1) - xd.sum(1) ** 2
    th_full = (0.5 * t).astype(np.float32)

    w2_dev = _chunk_major(w2).astype(BF16)
    wf_dev = _chunk_major(wf).astype(F8)
    w1_dev = _chunk_major(w1).astype(F8)
    # mw[i].T is the lhsT; layer-major, then chunk-major within each layer
    mwT = mw.transpose(0, 2, 1)  # [4, D(k), D(m)]
    mw_dev = np.ascontiguousarray(
        mwT.reshape(4, KC, P, D).transpose(2, 0, 1, 3).reshape(P, 4 * KC * D)
    ).astype(F8)
    # bias rows [mb0|mb1|mb2|btot] for the rank-1 matmuls (natural feature
    # order: output chunk jc partition p is feature jc*128+p)
    btot = (bias + mb[3]).astype(np.float32)
    bt_row = np.concatenate([mb[0], mb[1], mb[2], btot]).reshape(1, 4 * D)
    # per-layer MLP biases as per-partition columns: [mb0(4) | mb1(4) | mb2(4)]
    mb3 = mb[:3].astype(np.float32).reshape(3, KC, P).transpose(2, 0, 1).reshape(P, 12)
    bias_dev = np.concatenate([np.zeros((P, 4), np.float32), mb3], axis=1)

    in_maps = []
    for c in range(NCORES):
        xs = x[c * BL : (c + 1) * BL, :].T  # [512, 64]
        x_dev = np.ascontiguousarray(
            xs.reshape(KC, P, BL).transpose(1, 0, 2).reshape(P, KC * BL)
        )
        thb_dev = np.concatenate(
            [
                np.broadcast_to(th_full[c * BL : (c + 1) * BL], (P, BL)),
                bias_dev,
            ],
            axis=1,
        )
        # byte-pack: x8 fp8 | xb bf16 | th f32 | bias f32 | wf fp8
        act_dev = np.ascontiguousarray(
            np.concatenate(
                [
                    x_dev.astype(F8).view(np.uint8),
                    x_dev.astype(BF16).view(np.uint8),
                    thb_dev.astype(np.float32).view(np.uint8),
                    wf_dev.view(np.uint8),
                ],
                axis=1,
            )
        )
        in_maps.append(
            {
                "act_d": act_dev,
                "bt_d": bt_row.astype(F8),
                "w1_d": w1_dev,
                "mw_d": mw_dev,
                "w2_d": w2_dev,
            }
        )
    return in_maps


def assemble_output(results):
    out = np.empty((B, D), np.float32)
    for c in range(NCORES):
        od = results[c]["out_d"]  # [128, KC*BL]
        outT = od.reshape(P, KC, BL).transpose(1, 0, 2).reshape(D, BL)
        out[c * BL : (c + 1) * BL, :] = outT.T
    return out


def kernel(**inputs):
    from concourse.bass_utils import run_bass_kernel_spmd

    nc = _get_nc()
    in_maps = prepare_in_maps(inputs)
    res = run_bass_kernel_spmd(nc, in_maps, core_ids=list(range(NCORES)))
    return assemble_output(res.results)


# revision 54
# speedup vs baseline: 1.1179x; 1.0705x over previous
"""ContinuousDeepFM Trainium2 kernel (8-core data-parallel over batch).

Math (algebraically collapsed from the reference — the [B,D,D] interaction
tensor is never materialized):
    fo  = x @ W1 + bias
    xw  = x @ W2
    so[b,j] = 0.5 * xw[b,j]^2 * t[b],  t[b] = sum_i x[b,i]^2 - (sum_i x[b,i])^2
    h   = MLP(x @ Wf)   (3 ReLU layers + final linear, weights mlp_w[i].T)
    out = fo + so + h

Sharding: batch 512 -> 64 rows per core; weights replicated. On-chip layout
is feature-major (activations stored transposed as 4 chunks of 128
partitions) so no on-chip transposes are needed. t depends only on x, so it
is computed host-side in fp64 and shipped pre-broadcast.

Precision: the output is dominated by the second-order term (RMS ~2e5 vs
~23 for fo and ~1 for h). W2 and the x used with it run in bf16 (end-to-end
rel err 2.7e-3 vs the 2e-2 gate); the deep/first-order weights and
activations run in fp8e4m3; x is pre-cast host-side to both dtypes. All
biases ship as fp8 rows and enter each layer's PSUM via rank-1 matmuls
against a ones vector (~50ns each; fp32 matmuls would LOW/HIGH double-pass
at ~750ns), so drains are bias-free.

Schedule (from NTFF trace iteration):
- Both HWDGE rings share one pool of 16 DMA engines; ring choice is about
  ORDER/completion-gating, not bandwidth. >8 total DMAs round-robin onto 8
  semaphore lanes and serialize, so the input side is exactly 8 DMAs.
- x8|xb|th|bias|wf byte-packed into one 3136B-row tensor (h0's entire gate
  is one early DMA; thin-row tensors starve behind fat ones, so they must
  not sit mid-ring). Ring A: actwf, mw1, mw2, w1; ring B: bt, mw0 (l0
  ungates right after h0), w2, mw3 — per-layer mw DMAs land just ahead of
  their layer; each ring's last DMA feeds only tail work (the straggler
  DMA engine adds ~0.5-1us to every completion).
- Each PSUM accumulation group is ONE [128,256] tile in one bank (start
  zeroes the whole bank, so only a group's first matmul carries it) and
  matmuls are emitted in a diagonal (kc+jc) wavefront: output slices finish
  early, input chunks are first read late, and layer drains overlap the
  group. Drains are single wide Vector ReLU ops — the Tile scheduler
  chains multi-op drains ~320ns apart via transitive semaphore elision.
- xw + its square (Scalar) + *t (GpSimd) run mid-chain; the final PSUM
  group (btot + x@W1 + h3@mw3) closes right after the last weights land;
  out = o + so as two half adds, each half DMA'd on its own ring.
"""

import numpy as np
import ml_dtypes

B = 512
D = 512
NCORES = 8
BL = B // NCORES  # 64 batch rows per core
P = 128
KC = D // P  # 4 partition chunks of the feature dim

F8 = ml_dtypes.float8_e4m3
BF16 = ml_dtypes.bfloat16

_NC_CACHE = {}


def _split_multi_waits(nc, mybir):
    """This container's walrus build supports only ONE sync wait per
    instruction, but Tile's scheduler attaches several (e.g. the exit
    drain). Split extras into preceding single-wait NoOps on the same
    engine — in-order execution preserves the barrier semantics."""
    ctr = 0
    for fn in nc.m.functions:
        for blk in fn.blocks:
            insts = blk.instructions
            if not any(
                i.sync_info is not None
                and i.sync_info.on_wait
                and len(i.sync_info.on_wait) > 1
                for i in insts
            ):
                continue
            out = []
            for inst in insts:
                si = inst.sync_info
                if si is not None and si.on_wait and len(si.on_wait) > 1:
                    waits = list(si.on_wait)
                    for w in waits[:-1]:
                        ctr += 1
                        nop = mybir.InstNoOp(
                            name=f"wsplit-{ctr}-{inst.name}", ins=[], outs=[]
                        )
                        nop.engine = inst.engine
                        nop.sync_info = mybir.SyncInfo(on_wait=[w], on_update=[])
                        out.append(nop)
                    si.on_wait = [waits[-1]]
                out.append(inst)
            blk.instructions = out
    return ctr


def _build_nc():
    import concourse.bass as bass
    import concourse.mybir as mybir
    import concourse.tile as tile

    dt = mybir.dt
    f32 = dt.float32
    f8 = dt.float8e4
    bf = dt.bfloat16
    Alu = mybir.AluOpType
    Act = mybir.ActivationFunctionType

    nc = bass.Bass("TRN2", target_bir_lowering=False, debug=False)

    # x8 (fp8), xb (bf16), th+bias (f32) byte-packed into one tensor so the
    # whole activation side is ONE DMA: the tile framework round-robins DMAs
    # over 8 HWDGE semaphore lanes, and a 9th+ input DMA serializes behind
    # an earlier one's completion.
    # x8|xb|th|bias|wf packed into one 3136B-row tensor: h0 gates on this
    # single early DMA, and the rows are fat enough to stream well
    ACT_BYTES = KC * BL + 2 * KC * BL + 4 * (BL + 16) + KC * D
    act_d = nc.dram_tensor("act_d", [P, ACT_BYTES], dt.uint8, kind="ExternalInput")
    # ALL biases as fp8 rows [mb0|mb1|mb2|btot] (values ~±0.04; fp8
    # quantization error ~2e-3 absolute is invisible next to the ~2e5-RMS
    # output). They enter each layer's PSUM via rank-1 fp8 matmuls (~50ns
    # each), so the ReLU drains carry no bias and collapse to two wide ops
    # per layer — the Tile scheduler chains drains ~320ns apart, so fewer
    # drain ops directly shortens every layer transition.
    bt_d = nc.dram_tensor("bt_d", [1, 4 * D], f8, kind="ExternalInput")
    w1_d = nc.dram_tensor("w1_d", [P, KC * D], f8, kind="ExternalInput")
    mw_d = nc.dram_tensor("mw_d", [P, 4 * KC * D], f8, kind="ExternalInput")
    w2_d = nc.dram_tensor("w2_d", [P, KC * D], bf, kind="ExternalInput")
    out_d = nc.dram_tensor("out_d", [P, KC * BL], f32, kind="ExternalOutput")

    with tile.TileContext(nc) as tc:
        with (
            tc.tile_pool(name="w", bufs=1) as wpool,
            tc.tile_pool(name="act", bufs=1) as apool,
            tc.tile_pool(name="ps", bufs=1, space="PSUM") as pspool,
        ):
            # ---- input DMAs. Ring A (sync) carries the deep-chain weights
            # in consumption order; ring B (scalar) carries activations +
            # w2/w1. The rings share DMA engines, so this is ordering only.
            # Ring A (sync): act+wf packed (h0's whole gate), then mw1,
            # mw2, w1 in consumption order. Ring B (scalar): bt, mw0 (l0
            # ungates right after h0), w2, mw3.
            act_sb = apool.tile([P, ACT_BYTES], dt.uint8, tag="act")
            nc.sync.dma_start(act_sb[:], act_d.ap())
            mw_sb = wpool.tile([P, 4 * KC * D], f8, tag="mw")
            for i in (1, 2):
                nc.sync.dma_start(
                    mw_sb[:, i * KC * D : (i + 1) * KC * D],
                    mw_d.ap()[:, i * KC * D : (i + 1) * KC * D],
                )
            # the two tail-gating weights (w1, mw3) stream as HALF DMAs:
            # the diagonal matmul order reads kc0/kc1 first, so each final
            # group starts as soon as its first half lands
            w1_sb = wpool.tile([P, KC * D], f8, tag="w1")
            HW = KC * D // 2
            nc.sync.dma_start(w1_sb[:, 0:HW], w1_d.ap()[:, 0:HW])
            nc.sync.dma_start(w1_sb[:, HW : 2 * HW], w1_d.ap()[:, HW : 2 * HW])

            bt_sb = apool.tile([1, 4 * D], f8, tag="bt")
            nc.scalar.dma_start(bt_sb[:], bt_d.ap())
            nc.scalar.dma_start(
                mw_sb[:, 0 : KC * D], mw_d.ap()[:, 0 : KC * D]
            )
            w2_sb = wpool.tile([P, KC * D], bf, tag="w2")
            nc.scalar.dma_start(w2_sb[:], w2_d.ap())
            MW3 = 3 * KC * D
            nc.scalar.dma_start(
                mw_sb[:, MW3 : MW3 + HW], mw_d.ap()[:, MW3 : MW3 + HW]
            )
            nc.scalar.dma_start(
                mw_sb[:, MW3 + HW : MW3 + 2 * HW],
                mw_d.ap()[:, MW3 + HW : MW3 + 2 * HW],
            )

            # typed views into the packed tile (byte offsets: x8 @ 0,
            # xb @ 256, th @ 768, bias @ 1024, wf @ 1088)
            act_f8 = act_sb.bitcast(f8)
            act_bf = act_sb.bitcast(bf)
            act_f32 = act_sb.bitcast(f32)
            XB0 = KC * BL // 2  # xb start in bf16 elements
            TH0 = 3 * KC * BL // 4  # th start in f32 elements
            WF0 = KC * BL + 2 * KC * BL + 4 * (BL + 16)  # wf byte offset

            def th_ap():
                return act_f32[:, TH0 : TH0 + BL]

            def x8sl(kc):
                return act_f8[:, kc * BL : (kc + 1) * BL]

            def xbsl(kc):
                return act_bf[:, XB0 + kc * BL : XB0 + (kc + 1) * BL]

            # ones vector for the rank-1 bias matmuls
            ones = apool.tile([1, BL], f8, tag="ones")
            nc.gpsimd.memset(ones[:], 1.0)

            def wsl(t, kc, jc, base=0):
                return t[:, base + kc * D + jc * P : base + kc * D + (jc + 1) * P]

            def xsl(t, kc):
                return t[:, kc * BL : (kc + 1) * BL]

            # Diagonal wavefront (kc+jc ascending): input chunk kc is first
            # read at instruction ~kc(kc+1)/2 (late), while output slice jc
            # completes at instruction ~10+jc*3 (early) — so each layer's
            # drains overlap the group and the next layer never bubbles on
            # a drain that hasn't fired.
            DIAG = [
                (kc, w - kc)
                for w in range(2 * KC - 1)
                for kc in range(max(0, w - KC + 1), min(KC, w + 1))
            ]

            def mm_group(ps, w_t, rhs_fn, base=0, start=True, stop=True):
                # The group shares ONE PSUM bank: start zeroes the whole
                # bank, so only the group's first matmul may carry it (the
                # rest accumulate onto the zeroed bank); stop only on last.
                for n, (kc, jc) in enumerate(DIAG):
                    nc.tensor.matmul(
                        xsl(ps, jc),
                        wsl(w_t, kc, jc, base=base),
                        rhs_fn(kc),
                        start=start and n == 0,
                        stop=stop and n == len(DIAG) - 1,
                    )

            def psum_group(name):
                # one bank per group: [128, 256] f32 = 1KB/partition
                return pspool.tile([P, KC * BL], f32, tag="mm", bufs=8, name=name)

            # ---- deep chain (fp8): h0 = x @ Wf; single fused drain
            h_ps = psum_group("h0")
            mm_group(h_ps, act_f8, x8sl, base=WF0)
            h = apool.tile([P, KC * BL], f8, tag="h0")
            nc.vector.tensor_copy(h[:], h_ps[:])

            def bias_mms(ps, row, start=False):
                # rank-1 fp8 matmuls: add bias row `row` of bt_sb into every
                # slice of the group's PSUM bank before the weight matmuls
                for jc in range(KC):
                    nc.tensor.matmul(
                        xsl(ps, jc),
                        bt_sb[0:1, row * D + jc * P : row * D + (jc + 1) * P],
                        ones[0:1, :],
                        start=start and jc == 0,
                        stop=False,
                    )

            # drains are bias-free (bias lives in PSUM): ONE wide ReLU op
            # on Vector per layer — every extra drain op adds a ~320ns
            # scheduler-chained hop to the layer transition
            def relu_drain(ps, i):
                hn = apool.tile([P, KC * BL], f8, tag=f"h{i + 1}")
                nc.vector.tensor_scalar(hn[:], ps[:], 0.0, None, op0=Alu.max)
                return hn

            l_ps = psum_group("l0")
            bias_mms(l_ps, 0, start=True)
            mm_group(l_ps, mw_sb, lambda kc: xsl(h, kc), base=0, start=False)
            h1 = relu_drain(l_ps, 0)

            l_ps = psum_group("l1")
            bias_mms(l_ps, 1, start=True)
            mm_group(l_ps, mw_sb, lambda kc: xsl(h1, kc), base=KC * D,
                     start=False)
            h2 = relu_drain(l_ps, 1)

            # ---- xw = x @ W2 (bf16) slots behind l1 (w2 lands mid-chain).
            # Square in ONE Scalar op, *th on GpSimd — all done long before
            # the tail.
            xw_ps = psum_group("xw")
            mm_group(xw_ps, w2_sb, xbsl)
            xwsq = apool.tile([P, KC * BL], f32, tag="xwsq")
            nc.scalar.square(xwsq[:], xw_ps[:])
            so = apool.tile([P, KC * BL], f32, tag="so")
            for jc in range(KC):
                nc.gpsimd.tensor_mul(xsl(so, jc), xsl(xwsq, jc), th_ap())




            l_ps = psum_group("l2")
            bias_mms(l_ps, 2, start=True)
            mm_group(l_ps, mw_sb, lambda kc: xsl(h2, kc), base=2 * KC * D,
                     start=False)
            h3 = relu_drain(l_ps, 2)

            # ---- final PSUM group: btot + x@W1, then += h3 @ mw[3].T
            o_ps = psum_group("o")
            bias_mms(o_ps, 3, start=True)
            mm_group(o_ps, w1_sb, x8sl, start=False, stop=False)
            mm_group(o_ps, mw_sb, lambda kc: xsl(h3, kc), base=3 * KC * D,
                     start=False, stop=True)

            # out = o + so as two half adds, each half DMA'd on its own
            # ring as soon as its add lands
            out_sb = apool.tile([P, KC * BL], f32, tag="out")
            HO = KC * BL // 2
            nc.vector.tensor_add(out_sb[:, 0:HO], o_ps[:, 0:HO], so[:, 0:HO])
            nc.scalar.dma_start(out_d.ap()[:, 0:HO], out_sb[:, 0:HO])
            nc.vector.tensor_add(
                out_sb[:, HO : 2 * HO], o_ps[:, HO : 2 * HO], so[:, HO : 2 * HO]
            )
            nc.sync.dma_start(out_d.ap()[:, HO : 2 * HO], out_sb[:, HO : 2 * HO])

    _split_multi_waits(nc, mybir)
    return nc


def _get_nc():
    if "nc" not in _NC_CACHE:
        _NC_CACHE["nc"] = _build_nc()
    return _NC_CACHE["nc"]


def _chunk_major(w):
    """[D, D] lhsT-layout weight -> dense [128, KC*D] chunk-major array."""
    return np.ascontiguousarray(
        w.reshape(KC, P, D).transpose(1, 0, 2).reshape(P, KC * D)
    )


def prepare_in_maps(inputs):
    x = np.asarray(inputs["x"], np.float32)
    w1 = np.asarray(inputs["first_order_weights"], np.float32)
    bias = np.asarray(inputs["bias"], np.float32)
    w2 = np.asarray(inputs["second_order_weights"], np.float32)
    wf = np.asarray(inputs["feature_weights"], np.float32)
    mw = np.asarray(inputs["mlp_w"], np.float32)
    mb = np.asarray(inputs["mlp_b"], np.float32)

    # t[b] = sum x^2 - (sum x)^2 (host, fp64), shipped as 0.5*t broadcast
    xd = x.astype(np.float64)
    t = (xd * xd).sum(1) - xd.sum(1) ** 2
    th_full = (0.5 * t).astype(np.float32)

    w2_dev = _chunk_major(w2).astype(BF16)
    wf_dev = _chunk_major(wf).astype(F8)
    w1_dev = _chunk_major(w1).astype(F8)
    # mw[i].T is the lhsT; layer-major, then chunk-major within each layer
    mwT = mw.transpose(0, 2, 1)  # [4, D(k), D(m)]
    mw_dev = np.ascontiguousarray(
        mwT.reshape(4, KC, P, D).transpose(2, 0, 1, 3).reshape(P, 4 * KC * D)
    ).astype(F8)
    # bias rows [mb0|mb1|mb2|btot] for the rank-1 matmuls (natural feature
    # order: output chunk jc partition p is feature jc*128+p)
    btot = (bias + mb[3]).astype(np.float32)
    bt_row = np.concatenate([mb[0], mb[1], mb[2], btot]).reshape(1, 4 * D)
    # per-layer MLP biases as per-partition columns: [mb0(4) | mb1(4) | mb2(4)]
    mb3 = mb[:3].astype(np.float32).reshape(3, KC, P).transpose(2, 0, 1).reshape(P, 12)
    bias_dev = np.concatenate([np.zeros((P, 4), np.float32), mb3], axis=1)

    in_maps = []
    for c in range(NCORES):
        xs = x[c * BL : (c + 1) * BL, :].T  # [512, 64]
        x_dev = np.ascontiguousarray(
            xs.reshape(KC, P, BL).transpose(1, 0, 2).reshape(P, KC * BL)
        )
        thb_dev = np.concatenate(
            [
                np.broadcast_to(th_full[c * BL : (c + 1) * BL], (P, BL)),
                bias_dev,
            ],
            axis=1,
        )
        # byte-pack: x8 fp8 | xb bf16 | th f32 | bias f32 | wf fp8
        act_dev = np.ascontiguousarray(
            np.concatenate(
                [
                    x_dev.astype(F8).view(np.uint8),
                    x_dev.astype(BF16).view(np.uint8),
                    thb_dev.astype(np.float32).view(np.uint8),
                    wf_dev.view(np.uint8),
                ],
                axis=1,
            )
        )
        in_maps.append(
            {
                "act_d": act_dev,
                "bt_d": bt_row.astype(F8),
                "w1_d": w1_dev,
                "mw_d": mw_dev,
                "w2_d": w2_dev,
            }
        )
    return in_maps


def assemble_output(results):
    out = np.empty((B, D), np.float32)
    for c in range(NCORES):
        od = results[c]["out_d"]  # [128, KC*BL]
        outT = od.reshape(P, KC, BL).transpose(1, 0, 2).reshape(D, BL)
        out[c * BL : (c + 1) * BL, :] = outT.T
    return out


def kernel(**inputs):
    from concourse.bass_utils import run_bass_kernel_spmd

    nc = _get_nc()
    in_maps = prepare_in_maps(inputs)
    res = run_bass_kernel_spmd(nc, in_maps, core_ids=list(range(NCORES)))
    return assemble_output(res.results)
